# revision 7
# baseline (speedup 1.0000x reference)
"""Sliding-window attention (WINDOW=129) Trainium2 Bass kernel.

Problem: x[B=2, N=2048, C=768] -> qkv proj -> 12-head sliding-window
attention (half-window 64) -> output proj + bias.

Sharding: sequence-parallel over 8 cores: core c handles batch b = c//4,
query chunk s = c%4 (512 queries), with a 64-row halo each side for K/V.
Weights replicated. Each core computes its 512 output rows completely;
host concatenates. No collectives.

Per-core layout (all matmul operands fp16, psum f32):
  xT   [768, 640]   x-chunk+halo transposed (zero-padded outside seq)
  wqkT [768, 1536]  w_qkv[:1536].T, q-part pre-scaled by D**-0.5
  wvT  [768, 768]   w_qkv[1536:].T
  wpT  [768, 768]   w_proj.T
  bias [1, 768]
  maskT[5, 128, 256] per key-tile band/validity mask (0/1)

Pipeline: qkT = w_qk^T x^T -> [e, n] layout; v = x w_v^T -> [n, e] layout;
per key-tile kt: scores sT[k, q] via matmul (K=64, head pairs row-tiled),
exp on ACT (no max-subtraction: scores are N(0,1)-scale), 0/1 mask mult on
DVE; per query-tile r: AV via matmul with keys on contraction, fused
column-sum matmuls with a ones vector, reciprocal + gpsimd
partition-broadcast, normalize into attnT [c, n]; proj matmul + bias.
"""

import numpy as np

import concourse.bass as bass
import concourse.tile as tile
from concourse import bacc, mybir
from concourse._compat import with_exitstack

B, N, C = 2, 2048, 768
H, D = 12, 64
HALF = 64            # half window
NCORES = 8
CHUNK = 512          # queries per core
NK = CHUNK + 2 * HALF  # 640 rows incl halo
SCALE = D ** -0.5

F16 = mybir.dt.float16
F32 = mybir.dt.float32


@with_exitstack
def attn_core_kernel(ctx, tc, outs, ins):
    nc = tc.nc
    out_ap = outs["out"]
    xT, wqkT, wvT, wpT, bias, maskT = (
        ins["xT"], ins["wqkT"], ins["wvT"], ins["wpT"], ins["bias"], ins["maskT"],
    )

    consts = ctx.enter_context(tc.tile_pool(name="consts", bufs=1))
    ppool = ctx.enter_context(tc.tile_pool(name="ps", bufs=2, space="PSUM"))
    ptpool = ctx.enter_context(tc.tile_pool(name="pt", bufs=8))
    rbpool = ctx.enter_context(tc.tile_pool(name="rb", bufs=4))
    outpool = ctx.enter_context(tc.tile_pool(name="ob", bufs=2))
    dpool = ctx.enter_context(tc.tile_pool(name="dram", bufs=4, space="DRAM"))

    xT_sb = consts.tile([128, 6, NK], F16)
    wqk_sb = consts.tile([128, 6, 1536], F16)
    wv_sb = consts.tile([128, 6, 768], F16)
    wp_sb = consts.tile([128, 6, 768], F16)
    mask_sb = consts.tile([128, 5, 256], F16)
    bias_sb = consts.tile([128, 768], F32)
    qk_sb = consts.tile([64, 24, NK], F16)      # [64=d, head-group, n] q: h, k: 12+h
    v_sb = consts.tile([128, 5, 768], F16)      # [n-tile, e_v]
    attnT_sb = consts.tile([128, 6, CHUNK], F16)  # [c-tile, n]
    ones_sb = consts.tile([128, 1], F16)

    # ---- loads ----
    xT3 = xT.rearrange("(t p) n -> p t n", p=128)
    wqk3 = wqkT.rearrange("(t p) e -> p t e", p=128)
    wv3 = wvT.rearrange("(t p) e -> p t e", p=128)
    wp3 = wpT.rearrange("(t p) e -> p t e", p=128)
    for t in range(6):
        nc.sync.dma_start(xT_sb[:, t, :], xT3[:, t, :])
        nc.sync.dma_start(wqk_sb[:, t, :], wqk3[:, t, :])
        nc.sync.dma_start(wv_sb[:, t, :], wv3[:, t, :])
        nc.sync.dma_start(wp_sb[:, t, :], wp3[:, t, :])
    for kt in range(5):
        nc.sync.dma_start(mask_sb[:, kt, :], maskT[kt])
    nc.sync.dma_start(bias_sb[:], bias[0:1, :].to_broadcast((128, 768)))
    nc.vector.memset(ones_sb[:], 1.0)

    # ---- qkT: [e, n] = wqk^T.T @ xT, M=64 per head-group so every
    # head's rows live at SBUF partition base 0 (base-64 matmul operands
    # fault the device) ----
    for et in range(24):
        for c0, w in ((0, 512), (512, 128)):
            ps = ppool.tile([128, 512], F32, tag="mm")
            for ct in range(6):
                nc.tensor.matmul(
                    ps[0:64, :w],
                    wqk_sb[:, ct, et * 64:(et + 1) * 64],
                    xT_sb[:, ct, c0:c0 + w],
                    start=(ct == 0), stop=(ct == 5),
                )
            nc.vector.tensor_copy(out=qk_sb[:, et, c0:c0 + w], in_=ps[0:64, :w])

    # ---- v: [n, e_v] = xT.T @ wvT ----
    for nt in range(5):
        for c0, w in ((0, 512), (512, 256)):
            ps = ppool.tile([128, 512], F32, tag="mm")
            for ct in range(6):
                nc.tensor.matmul(
                    ps[:, :w],
                    xT_sb[:, ct, nt * 128:(nt + 1) * 128],
                    wv_sb[:, ct, c0:c0 + w],
                    start=(ct == 0), stop=(ct == 5),
                )
            nc.vector.tensor_copy(out=v_sb[:, nt, c0:c0 + w], in_=ps[:, :w])

    # ---- attention, pipelined over key tiles ----
    pt_tiles = {}

    def scores_kt(kt):
        # cq range actually consumed downstream
        cq0, cq1 = (128, 256) if kt == 0 else ((0, 128) if kt == 4 else (0, 256))
        for hq in range(3):
            sc = ppool.tile([128, 1024], F32, tag="sc")
            for j4 in range(4):
                h = 4 * hq + j4
                hp, j = h // 2, h % 2
                lhsT = qk_sb[:, 12 + h, kt * 128:kt * 128 + 128]
                rhs = qk_sb[:, h, kt * 128 - 64 + cq0:kt * 128 - 64 + cq1]
                nc.tensor.matmul(sc[:, 256 * j4 + cq0:256 * j4 + cq1], lhsT, rhs,
                                 start=True, stop=True)
            pt = ptpool.tile([128, 1024], F16, tag="pt")
            sc4 = sc.rearrange("p (h q) -> p h q", h=4)
            pt4 = pt.rearrange("p (h q) -> p h q", h=4)
            nc.scalar.activation(out=pt4[:, :, cq0:cq1], in_=sc4[:, :, cq0:cq1],
                                 func=mybir.ActivationFunctionType.Exp)
            nc.vector.tensor_tensor(
                pt4[:, :, cq0:cq1], pt4[:, :, cq0:cq1],
                mask_sb[:, kt:kt + 1, cq0:cq1].to_broadcast((128, 4, cq1 - cq0)),
                mybir.AluOpType.mult,
            )
            pt_tiles[(kt, hq)] = pt

    def av_r(r):
        for hp in range(6):
            hq, jb = hp // 2, 2 * (hp % 2)
            av = ppool.tile([128, 512], F32, tag="av")
            av4 = av.rearrange("p (h q) -> p h q", h=4)
            for j in range(2):  # head within pair
                h = 2 * hp + j
                j4 = h - 4 * hq
                for ki, kt in ((0, r), (1, r + 1)):
                    col0 = 128 if ki == 0 else 0
                    rhs = pt_tiles[(kt, hq)][:, 256 * j4 + col0:256 * j4 + col0 + 128]
                    lhsT = v_sb[:, kt, 64 * h:64 * h + 64]
                    nc.tensor.matmul(av[64 * j:64 * j + 64, 0:128], lhsT, rhs,
                                     start=(ki == 0), stop=(ki == 1))
            # column sums of both heads via ones vector
            for ki, kt in ((0, r), (1, r + 1)):
                col0 = 128 if ki == 0 else 0
                pt4 = pt_tiles[(kt, hq)].rearrange("p (h q) -> p h q", h=4)
                nc.tensor.matmul(av4[0:1, 2:4, :], ones_sb[:],
                                 pt4[:, jb:jb + 2, col0:col0 + 128],
                                 start=(ki == 0), stop=(ki == 1))
            qsl = slice(128 * r, 128 * r + 128)
            sums = rbpool.tile([1, 256], F32, tag="sums")
            nc.vector.tensor_copy(out=sums[0:1, :], in_=av4[0:1, 2:4, :])
            nc.vector.reciprocal(sums[0:1, :], sums[0:1, :])
            # broadcast [1, 128] -> [64, 128] per head via DRAM round-trip
            # (SBUF APs cannot have stride-0 partition dims; DRAM can)
            sums_d = dpool.tile([1, 256], F32, tag="sums_d")
            nc.sync.dma_start(sums_d[:], sums[:])
            rb = rbpool.tile([128, 128], F32, tag="rb")
            for j in range(2):
                nc.sync.dma_start(
                    rb[64 * j:64 * j + 64, :],
                    sums_d[0:1, 128 * j:128 * j + 128].to_broadcast((64, 128)),
                )
            nc.vector.tensor_tensor(attnT_sb[:, hp, qsl], av[:, 0:128], rb[:],
                                    mybir.AluOpType.mult)

    def proj_r(r):
        ob = outpool.tile([128, 768], F32, tag="ob")
        for c0, w in ((0, 512), (512, 256)):
            ps = ppool.tile([128, 512], F32, tag="mm")
            for ct in range(6):
                nc.tensor.matmul(
                    ps[:, :w],
                    attnT_sb[:, ct, 128 * r:128 * r + 128],
                    wp_sb[:, ct, c0:c0 + w],
                    start=(ct == 0), stop=(ct == 5),
                )
            nc.vector.tensor_add(out=ob[:, c0:c0 + w], in0=ps[:, :w],
                                 in1=bias_sb[:, c0:c0 + w])
        nc.sync.dma_start(out_ap[128 * r:128 * r + 128, :], ob[:])

    import os
    stage = int(os.environ.get("KSTAGE", "4"))
    if stage == 1:
        # debug: qkv only; dump v_sb
        for r in range(4):
            ob = outpool.tile([128, 768], F32, tag="ob")
            nc.vector.tensor_copy(out=ob[:], in_=v_sb[:, r, :])
            nc.sync.dma_start(out_ap[128 * r:128 * r + 128, :], ob[:])
        return
    if stage == 2:
        for kt in range(5):
            scores_kt(kt)
        for r in range(4):
            ob = outpool.tile([128, 768], F32, tag="ob")
            nc.vector.tensor_copy(out=ob[:], in_=v_sb[:, r, :])
            # consume pt tiles so nothing is dead
            nc.vector.tensor_add(out=ob[:, 0:512],
                                 in0=pt_tiles[(r, 0)][:, 0:512], in1=ob[:, 0:512])
            nc.sync.dma_start(out_ap[128 * r:128 * r + 128, :], ob[:])
        return
    scores_kt(0)
    scores_kt(1)
    for r in range(4):
        if r + 2 <= 4:
            scores_kt(r + 2)
        av_r(r)
        if stage == 3:
            ob = outpool.tile([128, 768], F32, tag="ob")
            nc.vector.tensor_copy(out=ob[:, 0:512],
                                  in_=attnT_sb[:, 0, 128 * r:128 * r + 128].to_broadcast((128, 512)) if False else attnT_sb[:, 0:4, 128 * r:128 * r + 128])
            nc.sync.dma_start(out_ap[128 * r:128 * r + 128, :], ob[:])
        else:
            proj_r(r)


def build_nc():
    nc = bacc.Bacc("TRN2", target_bir_lowering=False, debug=False)
    ins = {
        "xT": nc.dram_tensor("xT", [C, NK], F16, kind="ExternalInput").ap(),
        "wqkT": nc.dram_tensor("wqkT", [C, 2 * C], F16, kind="ExternalInput").ap(),
        "wvT": nc.dram_tensor("wvT", [C, C], F16, kind="ExternalInput").ap(),
        "wpT": nc.dram_tensor("wpT", [C, C], F16, kind="ExternalInput").ap(),
        "bias": nc.dram_tensor("bias", [1, C], F32, kind="ExternalInput").ap(),
        "maskT": nc.dram_tensor("maskT", [5, 128, 256], F16, kind="ExternalInput").ap(),
    }
    outs = {"out": nc.dram_tensor("out", [CHUNK, C], F32, kind="ExternalOutput").ap()}
    with tile.TileContext(nc) as tc:
        attn_core_kernel(tc, outs, ins)
    nc.finalize()
    return nc


def make_core_inputs(x, w_qkv, w_proj, b_proj):
    """Build the 8 per-core input maps from full inputs."""
    x = np.asarray(x, dtype=np.float32)
    w_qkv = np.asarray(w_qkv, dtype=np.float32)
    w_proj = np.asarray(w_proj, dtype=np.float32)
    b_proj = np.asarray(b_proj, dtype=np.float32)

    wqk = np.concatenate([w_qkv[:C] * SCALE, w_qkv[C:2 * C]], axis=0)
    wqkT = np.ascontiguousarray(wqk.T).astype(np.float16)
    wvT = np.ascontiguousarray(w_qkv[2 * C:].T).astype(np.float16)
    wpT = np.ascontiguousarray(w_proj.T).astype(np.float16)
    bias = b_proj.reshape(1, C).astype(np.float32)

    in_maps = []
    for c in range(NCORES):
        b, s = divmod(c, 4)
        lo = s * CHUNK - HALF
        hi = s * CHUNK + CHUNK + HALF
        xs = np.zeros((NK, C), dtype=np.float32)
        s0, s1 = max(lo, 0), min(hi, N)
        xs[s0 - lo:s1 - lo] = x[b, s0:s1]
        xT = np.ascontiguousarray(xs.T).astype(np.float16)

        mask = np.zeros((5, 128, 256), dtype=np.float16)
        k = np.arange(128)[:, None]
        cq = np.arange(256)[None, :]
        band = (cq - k >= 0) & (cq - k <= 128)
        for kt in range(5):
            key_seq = s * CHUNK - HALF + 128 * kt + k
            valid = (key_seq >= 0) & (key_seq < N)
            mask[kt] = (band & valid).astype(np.float16)

        in_maps.append({
            "xT": xT, "wqkT": wqkT, "wvT": wvT, "wpT": wpT,
            "bias": bias, "maskT": mask,
        })
    return in_maps


_NC_CACHE = None


def kernel(x, w_qkv, w_proj, b_proj):
    from concourse.bass_utils import run_bass_kernel_spmd

    global _NC_CACHE
    if _NC_CACHE is None:
        _NC_CACHE = build_nc()
    in_maps = make_core_inputs(x, w_qkv, w_proj, b_proj)
    res = run_bass_kernel_spmd(_NC_CACHE, in_maps, core_ids=list(range(NCORES)))
    out = np.empty((B, N, C), dtype=np.float32)
    for c in range(NCORES):
        b, s = divmod(c, 4)
        out[b, s * CHUNK:(s + 1) * CHUNK] = res.results[c]["out"]
    return out


# revision 9
# speedup vs baseline: 3.8147x; 3.8147x over previous
"""Sliding-window attention (WINDOW=129) Trainium2 Bass kernel.

Problem: x[B=2, N=2048, C=768] -> qkv proj -> 12-head sliding-window
attention (half-window 64) -> output proj + bias.

Sharding: sequence-parallel over 8 cores: core c handles batch b = c//4,
query chunk s = c%4 (512 queries), with a 64-row halo each side for K/V.
Weights replicated. Each core computes its 512 output rows completely;
host concatenates. No collectives.

Per-core layout (all matmul operands fp16, psum f32):
  xT   [768, 640]   x-chunk+halo transposed (zero-padded outside seq)
  wqkT [768, 1536]  w_qkv[:1536].T, q-part pre-scaled by D**-0.5
  wvT  [768, 768]   w_qkv[1536:].T
  wpT  [768, 768]   w_proj.T
  bias [1, 768]
  maskT[5, 128, 256] per key-tile band/validity mask (0/1)

Pipeline: qkT = w_qk^T x^T -> [e, n] layout; v = x w_v^T -> [n, e] layout;
per key-tile kt: scores sT[k, q] via matmul (K=64, head pairs row-tiled),
exp on ACT (no max-subtraction: scores are N(0,1)-scale), 0/1 mask mult on
DVE; per query-tile r: AV via matmul with keys on contraction, fused
column-sum matmuls with a ones vector, reciprocal + gpsimd
partition-broadcast, normalize into attnT [c, n]; proj matmul + bias.
"""

import numpy as np

import concourse.bass as bass
import concourse.tile as tile
from concourse import bacc, mybir
from concourse._compat import with_exitstack

B, N, C = 2, 2048, 768
H, D = 12, 64
HALF = 64            # half window
NCORES = 8
CHUNK = 512          # queries per core
NK = CHUNK + 2 * HALF  # 640 rows incl halo
SCALE = D ** -0.5

F16 = mybir.dt.float16
F32 = mybir.dt.float32


@with_exitstack
def attn_core_kernel(ctx, tc, outs, ins, repeat=1):
    nc = tc.nc
    out_ap = outs["out"]
    xT, wqkT, wvT, wpT, bias, maskT = (
        ins["xT"], ins["wqkT"], ins["wvT"], ins["wpT"], ins["bias"], ins["maskT"],
    )

    consts = ctx.enter_context(tc.tile_pool(name="consts", bufs=1))
    ppool = ctx.enter_context(tc.tile_pool(name="ps", bufs=2, space="PSUM"))
    ptpool = ctx.enter_context(tc.tile_pool(name="pt", bufs=8))
    rbpool = ctx.enter_context(tc.tile_pool(name="rb", bufs=4))
    outpool = ctx.enter_context(tc.tile_pool(name="ob", bufs=2))
    dpool = ctx.enter_context(tc.tile_pool(name="dram", bufs=4, space="DRAM"))

    xT_sb = consts.tile([128, 6, NK], F16)
    wqk_sb = consts.tile([128, 6, 1536], F16)
    wv_sb = consts.tile([128, 6, 768], F16)
    wp_sb = consts.tile([128, 6, 768], F16)
    mask_sb = consts.tile([128, 5, 256], F16)
    bias_sb = consts.tile([128, 768], F32)
    qk_sb = consts.tile([64, 24, NK], F16)      # [64=d, head-group, n] q: h, k: 12+h
    v_sb = consts.tile([128, 5, 768], F16)      # [n-tile, e_v]
    attnT_sb = consts.tile([128, 6, CHUNK], F16)  # [c-tile, n]
    ones_sb = consts.tile([128, 1], F16)

    # ---- loads ----
    xT3 = xT.rearrange("(t p) n -> p t n", p=128)
    wqk3 = wqkT.rearrange("(t p) e -> p t e", p=128)
    wv3 = wvT.rearrange("(t p) e -> p t e", p=128)
    wp3 = wpT.rearrange("(t p) e -> p t e", p=128)

    def loads():
        for t in range(6):
            nc.sync.dma_start(xT_sb[:, t, :], xT3[:, t, :])
            nc.sync.dma_start(wqk_sb[:, t, :], wqk3[:, t, :])
            nc.sync.dma_start(wv_sb[:, t, :], wv3[:, t, :])
            nc.sync.dma_start(wp_sb[:, t, :], wp3[:, t, :])
        for kt in range(5):
            nc.sync.dma_start(mask_sb[:, kt, :], maskT[kt])
        nc.sync.dma_start(bias_sb[:], bias[0:1, :].to_broadcast((128, 768)))
        nc.vector.memset(ones_sb[:], 1.0)

    def qkv():
        # qkT: [e, n] = wqk^T.T @ xT, M=64 per head-group so every head's
        # rows live at SBUF partition base 0 (base-64 matmul operands fault
        # the device)
        for et in range(24):
            for c0, w in ((0, 512), (512, 128)):
                ps = ppool.tile([128, 512], F32, tag="mm")
                for ct in range(6):
                    nc.tensor.matmul(
                        ps[0:64, :w],
                        wqk_sb[:, ct, et * 64:(et + 1) * 64],
                        xT_sb[:, ct, c0:c0 + w],
                        start=(ct == 0), stop=(ct == 5),
                    )
                nc.vector.tensor_copy(out=qk_sb[:, et, c0:c0 + w], in_=ps[0:64, :w])
        # v: [n, e_v] = xT.T @ wvT
        for nt in range(5):
            for c0, w in ((0, 512), (512, 256)):
                ps = ppool.tile([128, 512], F32, tag="mm")
                for ct in range(6):
                    nc.tensor.matmul(
                        ps[:, :w],
                        xT_sb[:, ct, nt * 128:(nt + 1) * 128],
                        wv_sb[:, ct, c0:c0 + w],
                        start=(ct == 0), stop=(ct == 5),
                    )
                nc.vector.tensor_copy(out=v_sb[:, nt, c0:c0 + w], in_=ps[:, :w])

    pt_tiles = {}

    def scores_kt(kt):
        # cq range actually consumed downstream
        cq0, cq1 = (128, 256) if kt == 0 else ((0, 128) if kt == 4 else (0, 256))
        for hq in range(3):
            sc = ppool.tile([128, 1024], F32, tag="sc")
            for j4 in range(4):
                h = 4 * hq + j4
                lhsT = qk_sb[:, 12 + h, kt * 128:kt * 128 + 128]
                rhs = qk_sb[:, h, kt * 128 - 64 + cq0:kt * 128 - 64 + cq1]
                nc.tensor.matmul(sc[:, 256 * j4 + cq0:256 * j4 + cq1], lhsT, rhs,
                                 start=True, stop=True)
            pt = ptpool.tile([128, 1024], F16, tag="pt")
            sc4 = sc.rearrange("p (h q) -> p h q", h=4)
            pt4 = pt.rearrange("p (h q) -> p h q", h=4)
            nc.scalar.activation(out=pt4[:, :, cq0:cq1], in_=sc4[:, :, cq0:cq1],
                                 func=mybir.ActivationFunctionType.Exp)
            nc.vector.tensor_tensor(
                pt4[:, :, cq0:cq1], pt4[:, :, cq0:cq1],
                mask_sb[:, kt:kt + 1, cq0:cq1].to_broadcast((128, 4, cq1 - cq0)),
                mybir.AluOpType.mult,
            )
            pt_tiles[(kt, hq)] = pt

    def av_r(r):
        for hp in range(6):
            hq, jb = hp // 2, 2 * (hp % 2)
            av = ppool.tile([128, 512], F32, tag="av")
            av4 = av.rearrange("p (h q) -> p h q", h=4)
            for j in range(2):  # head within pair
                h = 2 * hp + j
                j4 = h - 4 * hq
                for ki, kt in ((0, r), (1, r + 1)):
                    col0 = 128 if ki == 0 else 0
                    rhs = pt_tiles[(kt, hq)][:, 256 * j4 + col0:256 * j4 + col0 + 128]
                    lhsT = v_sb[:, kt, 64 * h:64 * h + 64]
                    nc.tensor.matmul(av[64 * j:64 * j + 64, 0:128], lhsT, rhs,
                                     start=(ki == 0), stop=(ki == 1))
            # column sums of both heads via ones vector
            for ki, kt in ((0, r), (1, r + 1)):
                col0 = 128 if ki == 0 else 0
                pt4 = pt_tiles[(kt, hq)].rearrange("p (h q) -> p h q", h=4)
                nc.tensor.matmul(av4[0:1, 2:4, :], ones_sb[:],
                                 pt4[:, jb:jb + 2, col0:col0 + 128],
                                 start=(ki == 0), stop=(ki == 1))
            qsl = slice(128 * r, 128 * r + 128)
            sums = rbpool.tile([1, 256], F32, tag="sums")
            nc.vector.tensor_copy(out=sums[0:1, :], in_=av4[0:1, 2:4, :])
            nc.vector.reciprocal(sums[0:1, :], sums[0:1, :])
            # broadcast [1, 128] -> [64, 128] per head via DRAM round-trip
            # (SBUF APs cannot have stride-0 partition dims; DRAM can)
            sums_d = dpool.tile([1, 256], F32, tag="sums_d")
            nc.sync.dma_start(sums_d[:], sums[:])
            rb = rbpool.tile([128, 128], F32, tag="rb")
            for j in range(2):
                nc.sync.dma_start(
                    rb[64 * j:64 * j + 64, :],
                    sums_d[0:1, 128 * j:128 * j + 128].to_broadcast((64, 128)),
                )
            nc.vector.tensor_tensor(attnT_sb[:, hp, qsl], av[:, 0:128], rb[:],
                                    mybir.AluOpType.mult)

    def proj_r(r):
        ob = outpool.tile([128, 768], F32, tag="ob")
        for c0, w in ((0, 512), (512, 256)):
            ps = ppool.tile([128, 512], F32, tag="mm")
            for ct in range(6):
                nc.tensor.matmul(
                    ps[:, :w],
                    attnT_sb[:, ct, 128 * r:128 * r + 128],
                    wp_sb[:, ct, c0:c0 + w],
                    start=(ct == 0), stop=(ct == 5),
                )
            nc.vector.tensor_add(out=ob[:, c0:c0 + w], in0=ps[:, :w],
                                 in1=bias_sb[:, c0:c0 + w])
        nc.sync.dma_start(out_ap[128 * r:128 * r + 128, :], ob[:])

    for _rep in range(repeat):
        pt_tiles.clear()
        loads()
        qkv()
        scores_kt(0)
        scores_kt(1)
        for r in range(4):
            if r + 2 <= 4:
                scores_kt(r + 2)
            av_r(r)
            proj_r(r)


def build_nc(repeat=1):
    nc = bacc.Bacc("TRN2", target_bir_lowering=False, debug=False)
    ins = {
        "xT": nc.dram_tensor("xT", [C, NK], F16, kind="ExternalInput").ap(),
        "wqkT": nc.dram_tensor("wqkT", [C, 2 * C], F16, kind="ExternalInput").ap(),
        "wvT": nc.dram_tensor("wvT", [C, C], F16, kind="ExternalInput").ap(),
        "wpT": nc.dram_tensor("wpT", [C, C], F16, kind="ExternalInput").ap(),
        "bias": nc.dram_tensor("bias", [1, C], F32, kind="ExternalInput").ap(),
        "maskT": nc.dram_tensor("maskT", [5, 128, 256], F16, kind="ExternalInput").ap(),
    }
    outs = {"out": nc.dram_tensor("out", [CHUNK, C], F32, kind="ExternalOutput").ap()}
    with tile.TileContext(nc) as tc:
        attn_core_kernel(tc, outs, ins, repeat=repeat)
    nc.finalize()
    return nc


def make_core_inputs(x, w_qkv, w_proj, b_proj):
    """Build the 8 per-core input maps from full inputs."""
    x = np.asarray(x, dtype=np.float32)
    w_qkv = np.asarray(w_qkv, dtype=np.float32)
    w_proj = np.asarray(w_proj, dtype=np.float32)
    b_proj = np.asarray(b_proj, dtype=np.float32)

    wqk = np.concatenate([w_qkv[:C] * SCALE, w_qkv[C:2 * C]], axis=0)
    wqkT = np.ascontiguousarray(wqk.T).astype(np.float16)
    wvT = np.ascontiguousarray(w_qkv[2 * C:].T).astype(np.float16)
    wpT = np.ascontiguousarray(w_proj.T).astype(np.float16)
    bias = b_proj.reshape(1, C).astype(np.float32)

    in_maps = []
    for c in range(NCORES):
        b, s = divmod(c, 4)
        lo = s * CHUNK - HALF
        hi = s * CHUNK + CHUNK + HALF
        xs = np.zeros((NK, C), dtype=np.float32)
        s0, s1 = max(lo, 0), min(hi, N)
        xs[s0 - lo:s1 - lo] = x[b, s0:s1]
        xT = np.ascontiguousarray(xs.T).astype(np.float16)

        mask = np.zeros((5, 128, 256), dtype=np.float16)
        k = np.arange(128)[:, None]
        cq = np.arange(256)[None, :]
        band = (cq - k >= 0) & (cq - k <= 128)
        for kt in range(5):
            key_seq = s * CHUNK - HALF + 128 * kt + k
            valid = (key_seq >= 0) & (key_seq < N)
            mask[kt] = (band & valid).astype(np.float16)

        in_maps.append({
            "xT": xT, "wqkT": wqkT, "wvT": wvT, "wpT": wpT,
            "bias": bias, "maskT": mask,
        })
    return in_maps


_NC_CACHE = None


def kernel(x, w_qkv, w_proj, b_proj):
    from concourse.bass_utils import run_bass_kernel_spmd

    global _NC_CACHE
    if _NC_CACHE is None:
        _NC_CACHE = build_nc()
    in_maps = make_core_inputs(x, w_qkv, w_proj, b_proj)
    res = run_bass_kernel_spmd(_NC_CACHE, in_maps, core_ids=list(range(NCORES)))
    out = np.empty((B, N, C), dtype=np.float32)
    for c in range(NCORES):
        b, s = divmod(c, 4)
        out[b, s * CHUNK:(s + 1) * CHUNK] = res.results[c]["out"]
    return out
